# revision 12
# baseline (speedup 1.0000x reference)
"""LoRA-linear Trainium2 Bass kernel (v3: bf16, pipelined, shared PSUM pool).

Computes, for T adapters: out[t] = x @ W.T + (x @ A_t.T) @ B_t.T + bias
Output: [T, B, S, Dout] float32.

Sharding: data-parallel over tokens across 8 NeuronCores (2048 tokens/core);
W/bias/selected-LoRA replicated. Matmul inputs are cast to bf16 on the host
(halves load traffic, enables fast-weight-load); accumulation stays fp32.

Per-core layout puts Dout on PSUM partitions (out.T tiles [dout=128, tok]):
  lowT[32t+j, tok] = sum_d A_t[j,d] x[tok,d]   (PE, k-major so compute starts
              on the first arriving x k-tile)
  base.T[m] = W[m-tile] @ x.T  (PE, accumulate over 8 k-tiles; bias folded
              into the ScalarE PSUM->SBUF evacuation as a per-partition bias)
  delta.T[t,m] = B_t.T[:, m-tile].T-contract lowT_t  (K=16 row-group matmuls)
  out.T[t,m] = base.T[m] + delta.T[t,m]  (VectorE tensor_tensor, PSUM+SBUF)
Delta matmuls for dout-tile m-1 are emitted after the base matmuls of tile m
(software pipelining) so the PE never stalls waiting for VectorE adds and the
HAM clock gate stays warm. Stores are contiguous 1MB blocks of out.T; the
host transposes back.
"""

import sys

if "/opt/trn_rl_repo" not in sys.path:
    sys.path.insert(0, "/opt/trn_rl_repo")

from contextlib import ExitStack

import ml_dtypes
import numpy as np

import concourse.bacc as bacc
import concourse.bass as bass
import concourse.mybir as mybir
import concourse.tile as tile
from concourse import bass_utils

# Problem constants (hardcoded per spec).
B, S, DIN, DOUT, R, NL, T = 4, 4096, 1024, 1024, 16, 8, 4
NCORES = 8
NTOK = B * S                 # 16384
CTOK = NTOK // NCORES        # 2048 tokens per core
KT = DIN // 128              # 8 k-tiles
MT = DOUT // 128             # 8 dout-tiles
NC_CHUNK = CTOK // 512       # 4 token-chunks of 512

F32 = mybir.dt.float32
BF16 = mybir.dt.bfloat16
NPBF16 = ml_dtypes.bfloat16


def _build_program():
    nc = bacc.Bacc("TRN2", target_bir_lowering=False, debug=False,
                   num_devices=NCORES)

    xt = nc.dram_tensor("xt", [DIN, CTOK], BF16, kind="ExternalInput").ap()
    wt = nc.dram_tensor("wt", [DIN, DOUT], BF16, kind="ExternalInput").ap()
    atp = nc.dram_tensor("atp", [DIN, 128], BF16, kind="ExternalInput").ap()
    btp = nc.dram_tensor("btp", [128, DOUT], BF16, kind="ExternalInput").ap()
    biasc = nc.dram_tensor("biasc", [128, MT], F32, kind="ExternalInput").ap()
    out = nc.dram_tensor("out", [T, MT, 128, CTOK], F32,
                         kind="ExternalOutput").ap()

    with tile.TileContext(nc) as tc, ExitStack() as ctx:
        const = ctx.enter_context(tc.tile_pool(name="const", bufs=1))
        base_sb = ctx.enter_context(tc.tile_pool(name="base_sb", bufs=3))
        out_sb = ctx.enter_context(tc.tile_pool(name="out_sb", bufs=2))
        bp_ps = ctx.enter_context(tc.tile_pool(name="bp_ps", bufs=2, space="PSUM"))
        dp_ps = ctx.enter_context(tc.tile_pool(name="dp_ps", bufs=6, space="PSUM"))

        # Loads: small tiles go on the scalar HWDGE ring (own FIFO, lands in
        # ~1us) so warm-up/phase-1 start immediately; the two big tensors are
        # single strided DMAs on the sync ring (one completion receipt each
        # instead of 16 serialized ones).
        at_all = const.tile([128, KT * 128], BF16, tag="at")
        nc.scalar.dma_start(at_all.rearrange("p (k r) -> p k r", k=KT),
                            atp.rearrange("(k p) r -> p k r", p=128))
        bt_s = const.tile([128, DOUT], BF16, tag="bt")
        nc.scalar.dma_start(bt_s[:], btp[:, :])
        bias_s = const.tile([128, MT], F32, tag="bias")
        nc.scalar.dma_start(bias_s[:], biasc[:, :])
        xt_all = const.tile([128, KT * CTOK], BF16, tag="xt")
        nc.sync.dma_start(xt_all.rearrange("p (k t) -> p k t", k=KT),
                          xt.rearrange("(k p) t -> p k t", p=128))
        wt_all = const.tile([128, KT * DOUT], BF16, tag="wt")
        nc.sync.dma_start(wt_all.rearrange("p (k o) -> p k o", k=KT),
                          wt.rearrange("(k p) o -> p k o", p=128))
        at_t = [at_all[:, bass.ts(k, 128)] for k in range(KT)]
        xt_t = [xt_all[:, bass.ts(k, CTOK)] for k in range(KT)]
        wt_t = [wt_all[:, bass.ts(k, DOUT)] for k in range(KT)]
        lowT_s = const.tile([128, CTOK], BF16, tag="lowT")

        # Warm-up matmuls during the DMA prologue: the HAM clock gate needs
        # ~3.4us of sustained PE activity to unthrottle 1.2 -> 2.4 GHz, and
        # the load-paced opening would otherwise run the whole first ~30us of
        # real matmuls at half clock. ~170 N=128 matmuls on already-resident
        # tiles span the ~16us prologue.
        warm = dp_ps.tile([128, 128], F32, tag="dp", name="warm")
        for _ in range(140):
            nc.tensor.matmul(warm[:], at_t[0][:], bt_s[:, 0:128],
                             start=True, stop=True)

        # Phase 1 (k-major): lowT[32t+j, tok] = sum_d A_sel[t,j,d] x[tok,d].
        lps = [dp_ps.tile([128, 512], F32, tag="dp", name=f"lp{c}")
               for c in range(NC_CHUNK)]
        for k in range(KT):
            for c in range(NC_CHUNK):
                nc.tensor.matmul(
                    lps[c][:],
                    at_t[k][:],
                    xt_t[k][:, bass.ts(c, 512)],
                    start=(k == 0), stop=(k == KT - 1),
                )
        for c in range(NC_CHUNK):
            nc.scalar.copy(lowT_s[:, bass.ts(c, 512)], lps[c][:])

        # Phase 2, software-pipelined chunk-wise: base(m) chunk c is emitted
        # before delta(m-1) chunk c so the PE always has a dense base group to
        # chew while VectorE drains the previous delta bank.
        def emit_base_chunk(m, bsb, c):
            bp = bp_ps.tile([128, 512], F32, tag="bp", name=f"bp{m}_{c}")
            for k in range(KT):
                nc.tensor.matmul(
                    bp[:],
                    wt_t[k][:, bass.ts(m, 128)],
                    xt_t[k][:, bass.ts(c, 512)],
                    start=(k == 0), stop=(k == KT - 1),
                )
            # Evacuate with the per-partition bias folded in.
            nc.scalar.activation(
                bsb[:, bass.ts(c, 512)], bp[:],
                mybir.ActivationFunctionType.Identity,
                bias=bias_s[:, m:m + 1],
            )

        def emit_delta_chunk(m, bsb, ods, c):
            for t in range(T):
                dp = dp_ps.tile([128, 512], F32, tag="dp", name=f"dp{m}_{c}_{t}")
                nc.tensor.matmul(
                    dp[:],
                    bt_s[32 * t:32 * t + R, bass.ts(m, 128)],
                    lowT_s[32 * t:32 * t + R, bass.ts(c, 512)],
                    start=True, stop=True,
                    tile_position=(32 * t, 0),
                )
                nc.vector.tensor_add(
                    ods[t][:, bass.ts(c, 512)],
                    bsb[:, bass.ts(c, 512)], dp[:],
                )

        def make_ods(m):
            return [out_sb.tile([128, CTOK], F32, tag=f"od{t}", name=f"od{t}_{m}")
                    for t in range(T)]

        def store_ods(m, ods):
            for t in range(T):
                nc.sync.dma_start(out[t, m, :, :], ods[t][:])

        prev_bsb = None
        prev_ods = None
        for m in range(MT):
            bsb = base_sb.tile([128, CTOK], F32, tag="bsb", name=f"bsb{m}")
            for c in range(NC_CHUNK):
                emit_base_chunk(m, bsb, c)
                if prev_bsb is not None:
                    emit_delta_chunk(m - 1, prev_bsb, prev_ods, c)
            if prev_ods is not None:
                store_ods(m - 1, prev_ods)
            prev_bsb, prev_ods = bsb, make_ods(m)
        for c in range(NC_CHUNK):
            emit_delta_chunk(MT - 1, prev_bsb, prev_ods, c)
        store_ods(MT - 1, prev_ods)

    nc.compile()
    return nc


_NC = None


def _get_program():
    global _NC
    if _NC is None:
        _NC = _build_program()
    return _NC


def kernel(**inputs):
    x = np.ascontiguousarray(np.asarray(inputs["x"], dtype=np.float32))
    W = np.asarray(inputs["W"], dtype=np.float32)
    bias_v = np.asarray(inputs["bias"], dtype=np.float32)
    lora_A = np.asarray(inputs["lora_A"], dtype=np.float32)
    lora_B = np.asarray(inputs["lora_B"], dtype=np.float32)
    tuner_index = np.asarray(inputs["tuner_index"]).astype(np.int64)

    assert x.shape == (B, S, DIN) and W.shape == (DOUT, DIN)
    assert tuner_index.shape == (T,)

    A_sel = lora_A[tuner_index]          # [T, R, Din]
    B_sel = lora_B[tuner_index]          # [T, Dout, R]

    xT = np.ascontiguousarray(x.reshape(NTOK, DIN).T).astype(NPBF16)
    wt = np.ascontiguousarray(W.T).astype(NPBF16)       # [Din, Dout]
    atp = np.zeros((DIN, 128), NPBF16)
    atp.reshape(DIN, T, 32)[:, :, :R] = A_sel.transpose(2, 0, 1).astype(NPBF16)
    btp = np.zeros((128, DOUT), NPBF16)
    btp.reshape(T, 32, DOUT)[:, :R, :] = B_sel.transpose(0, 2, 1).astype(NPBF16)
    biasc = np.ascontiguousarray(bias_v.reshape(MT, 128).T)   # [128, MT]

    in_maps = []
    for c in range(NCORES):
        in_maps.append({
            "xt": np.ascontiguousarray(xT[:, c * CTOK:(c + 1) * CTOK]),
            "wt": wt,
            "atp": atp,
            "btp": btp,
            "biasc": biasc,
        })

    nc = _get_program()
    res = bass_utils.run_bass_kernel_spmd(nc, in_maps, core_ids=list(range(NCORES)))

    big = np.empty((T, MT, 128, NTOK), np.float32)
    for c in range(NCORES):
        big[:, :, :, c * CTOK:(c + 1) * CTOK] = res.results[c]["out"]
    # [T, m, p, tok] -> [T, tok, m*128+p]
    full = np.ascontiguousarray(big.transpose(0, 3, 1, 2))
    return full.reshape(T, B, S, DOUT)


# revision 13
# speedup vs baseline: 1.1605x; 1.1605x over previous
"""LoRA-linear Trainium2 Bass kernel (v3: bf16, pipelined, shared PSUM pool).

Computes, for T adapters: out[t] = x @ W.T + (x @ A_t.T) @ B_t.T + bias
Output: [T, B, S, Dout] float32.

Sharding: data-parallel over tokens across 8 NeuronCores (2048 tokens/core);
W/bias/selected-LoRA replicated. Matmul inputs are cast to bf16 on the host
(halves load traffic, enables fast-weight-load); accumulation stays fp32.

Per-core layout puts Dout on PSUM partitions (out.T tiles [dout=128, tok]):
  lowT[32t+j, tok] = sum_d A_t[j,d] x[tok,d]   (PE, k-major so compute starts
              on the first arriving x k-tile)
  base.T[m] = W[m-tile] @ x.T  (PE, accumulate over 8 k-tiles; bias folded
              into the ScalarE PSUM->SBUF evacuation as a per-partition bias)
  delta.T[t,m] = B_t.T[:, m-tile].T-contract lowT_t  (K=16 row-group matmuls)
  out.T[t,m] = base.T[m] + delta.T[t,m]  (VectorE tensor_tensor, PSUM+SBUF)
Delta matmuls for dout-tile m-1 are emitted after the base matmuls of tile m
(software pipelining) so the PE never stalls waiting for VectorE adds and the
HAM clock gate stays warm. Stores are contiguous 1MB blocks of out.T; the
host transposes back.
"""

import sys

if "/opt/trn_rl_repo" not in sys.path:
    sys.path.insert(0, "/opt/trn_rl_repo")

from contextlib import ExitStack

import ml_dtypes
import numpy as np

import concourse.bacc as bacc
import concourse.bass as bass
import concourse.mybir as mybir
import concourse.tile as tile
from concourse import bass_utils

# Problem constants (hardcoded per spec).
B, S, DIN, DOUT, R, NL, T = 4, 4096, 1024, 1024, 16, 8, 4
NCORES = 8
NTOK = B * S                 # 16384
CTOK = NTOK // NCORES        # 2048 tokens per core
KT = DIN // 128              # 8 k-tiles
MT = DOUT // 128             # 8 dout-tiles
NC_CHUNK = CTOK // 512       # 4 token-chunks of 512

F32 = mybir.dt.float32
BF16 = mybir.dt.bfloat16
NPBF16 = ml_dtypes.bfloat16


def _build_program():
    nc = bacc.Bacc("TRN2", target_bir_lowering=False, debug=False,
                   num_devices=NCORES)

    xt = nc.dram_tensor("xt", [DIN, CTOK], BF16, kind="ExternalInput").ap()
    wt = nc.dram_tensor("wt", [DIN, DOUT], BF16, kind="ExternalInput").ap()
    atp = nc.dram_tensor("atp", [DIN, 128], BF16, kind="ExternalInput").ap()
    btp = nc.dram_tensor("btp", [128, DOUT], BF16, kind="ExternalInput").ap()
    biasc = nc.dram_tensor("biasc", [128, MT], F32, kind="ExternalInput").ap()
    out = nc.dram_tensor("out", [T, MT, 128, CTOK], F32,
                         kind="ExternalOutput").ap()

    with tile.TileContext(nc) as tc, ExitStack() as ctx:
        const = ctx.enter_context(tc.tile_pool(name="const", bufs=1))
        base_sb = ctx.enter_context(tc.tile_pool(name="base_sb", bufs=3))
        out_sb = ctx.enter_context(tc.tile_pool(name="out_sb", bufs=2))
        bp_ps = ctx.enter_context(tc.tile_pool(name="bp_ps", bufs=2, space="PSUM"))
        dp_ps = ctx.enter_context(tc.tile_pool(name="dp_ps", bufs=6, space="PSUM"))

        # Loads: small tiles go on the scalar HWDGE ring (own FIFO, lands in
        # ~1us) so warm-up/phase-1 start immediately; the two big tensors are
        # single strided DMAs on the sync ring (one completion receipt each
        # instead of 16 serialized ones).
        at_all = const.tile([128, KT * 128], BF16, tag="at")
        nc.scalar.dma_start(at_all.rearrange("p (k r) -> p k r", k=KT),
                            atp.rearrange("(k p) r -> p k r", p=128))
        bt_s = const.tile([128, DOUT], BF16, tag="bt")
        nc.scalar.dma_start(bt_s[:], btp[:, :])
        bias_s = const.tile([128, MT], F32, tag="bias")
        nc.scalar.dma_start(bias_s[:], biasc[:, :])
        at_t = [at_all[:, bass.ts(k, 128)] for k in range(KT)]
        xt_t = []
        wt_t = []
        for k in range(KT):
            tx = const.tile([128, CTOK], BF16, tag=f"xt{k}")
            nc.sync.dma_start(tx[:], xt[bass.ts(k, 128), :])
            xt_t.append(tx)
            tw = const.tile([128, DOUT], BF16, tag=f"wt{k}")
            nc.sync.dma_start(tw[:], wt[bass.ts(k, 128), :])
            wt_t.append(tw)
        lowT_s = const.tile([128, CTOK], BF16, tag="lowT")

        # Warm-up matmuls during the DMA prologue: the HAM clock gate needs
        # ~3.4us of sustained PE activity to unthrottle 1.2 -> 2.4 GHz, and
        # the load-paced opening would otherwise run the whole first ~30us of
        # real matmuls at half clock. ~170 N=128 matmuls on already-resident
        # tiles span the ~16us prologue.
        warm = dp_ps.tile([128, 128], F32, tag="dp", name="warm")
        for _ in range(140):
            nc.tensor.matmul(warm[:], at_t[0][:], bt_s[:, 0:128],
                             start=True, stop=True)

        # Phase 1 (k-major): lowT[32t+j, tok] = sum_d A_sel[t,j,d] x[tok,d].
        lps = [dp_ps.tile([128, 512], F32, tag="dp", name=f"lp{c}")
               for c in range(NC_CHUNK)]
        for k in range(KT):
            for c in range(NC_CHUNK):
                nc.tensor.matmul(
                    lps[c][:],
                    at_t[k][:],
                    xt_t[k][:, bass.ts(c, 512)],
                    start=(k == 0), stop=(k == KT - 1),
                )
        for c in range(NC_CHUNK):
            nc.scalar.copy(lowT_s[:, bass.ts(c, 512)], lps[c][:])

        # Phase 2, software-pipelined chunk-wise: base(m) chunk c is emitted
        # before delta(m-1) chunk c so the PE always has a dense base group to
        # chew while VectorE drains the previous delta bank.
        def emit_base_chunk(m, bsb, c):
            bp = bp_ps.tile([128, 512], F32, tag="bp", name=f"bp{m}_{c}")
            for k in range(KT):
                nc.tensor.matmul(
                    bp[:],
                    wt_t[k][:, bass.ts(m, 128)],
                    xt_t[k][:, bass.ts(c, 512)],
                    start=(k == 0), stop=(k == KT - 1),
                )
            # Evacuate with the per-partition bias folded in.
            nc.scalar.activation(
                bsb[:, bass.ts(c, 512)], bp[:],
                mybir.ActivationFunctionType.Identity,
                bias=bias_s[:, m:m + 1],
            )

        def emit_delta_chunk(m, bsb, ods, c):
            for t in range(T):
                dp = dp_ps.tile([128, 512], F32, tag="dp", name=f"dp{m}_{c}_{t}")
                nc.tensor.matmul(
                    dp[:],
                    bt_s[32 * t:32 * t + R, bass.ts(m, 128)],
                    lowT_s[32 * t:32 * t + R, bass.ts(c, 512)],
                    start=True, stop=True,
                    tile_position=(32 * t, 0),
                )
                nc.vector.tensor_add(
                    ods[t][:, bass.ts(c, 512)],
                    bsb[:, bass.ts(c, 512)], dp[:],
                )

        def make_ods(m):
            return [out_sb.tile([128, CTOK], F32, tag=f"od{t}", name=f"od{t}_{m}")
                    for t in range(T)]

        def store_ods(m, ods):
            for t in range(T):
                nc.sync.dma_start(out[t, m, :, :], ods[t][:])

        prev_bsb = None
        prev_ods = None
        for m in range(MT):
            bsb = base_sb.tile([128, CTOK], F32, tag="bsb", name=f"bsb{m}")
            for c in range(NC_CHUNK):
                emit_base_chunk(m, bsb, c)
                if prev_bsb is not None:
                    emit_delta_chunk(m - 1, prev_bsb, prev_ods, c)
            if prev_ods is not None:
                store_ods(m - 1, prev_ods)
            prev_bsb, prev_ods = bsb, make_ods(m)
        for c in range(NC_CHUNK):
            emit_delta_chunk(MT - 1, prev_bsb, prev_ods, c)
        store_ods(MT - 1, prev_ods)

    nc.compile()
    return nc


_NC = None


def _get_program():
    global _NC
    if _NC is None:
        _NC = _build_program()
    return _NC


def kernel(**inputs):
    x = np.ascontiguousarray(np.asarray(inputs["x"], dtype=np.float32))
    W = np.asarray(inputs["W"], dtype=np.float32)
    bias_v = np.asarray(inputs["bias"], dtype=np.float32)
    lora_A = np.asarray(inputs["lora_A"], dtype=np.float32)
    lora_B = np.asarray(inputs["lora_B"], dtype=np.float32)
    tuner_index = np.asarray(inputs["tuner_index"]).astype(np.int64)

    assert x.shape == (B, S, DIN) and W.shape == (DOUT, DIN)
    assert tuner_index.shape == (T,)

    A_sel = lora_A[tuner_index]          # [T, R, Din]
    B_sel = lora_B[tuner_index]          # [T, Dout, R]

    xT = np.ascontiguousarray(x.reshape(NTOK, DIN).T).astype(NPBF16)
    wt = np.ascontiguousarray(W.T).astype(NPBF16)       # [Din, Dout]
    atp = np.zeros((DIN, 128), NPBF16)
    atp.reshape(DIN, T, 32)[:, :, :R] = A_sel.transpose(2, 0, 1).astype(NPBF16)
    btp = np.zeros((128, DOUT), NPBF16)
    btp.reshape(T, 32, DOUT)[:, :R, :] = B_sel.transpose(0, 2, 1).astype(NPBF16)
    biasc = np.ascontiguousarray(bias_v.reshape(MT, 128).T)   # [128, MT]

    in_maps = []
    for c in range(NCORES):
        in_maps.append({
            "xt": np.ascontiguousarray(xT[:, c * CTOK:(c + 1) * CTOK]),
            "wt": wt,
            "atp": atp,
            "btp": btp,
            "biasc": biasc,
        })

    nc = _get_program()
    res = bass_utils.run_bass_kernel_spmd(nc, in_maps, core_ids=list(range(NCORES)))

    big = np.empty((T, MT, 128, NTOK), np.float32)
    for c in range(NCORES):
        big[:, :, :, c * CTOK:(c + 1) * CTOK] = res.results[c]["out"]
    # [T, m, p, tok] -> [T, tok, m*128+p]
    full = np.ascontiguousarray(big.transpose(0, 3, 1, 2))
    return full.reshape(T, B, S, DOUT)
